# revision 28
# baseline (speedup 1.0000x reference)
"""Two-layer GCN encoder (GCNConv x2 + minmax + L2 normalize) on 8 TRN2 NeuronCores.

Algebra: with A = D^-1/2 (Adj+I) D^-1/2 and no nonlinearity between the two
GCNConv layers, out = minmax_l2( A.(A.x).(W1@W2) + rowsumA.(b1@W2) + b2 ).
The symmetric norm factorizes: each aggregation is dinv[d] * sum_e t[src_e]
with the table pre-scaled by dinv (t1 = dinv*x on host, t2 = dinv^2*(A-sum)
on device), so the per-tile selection matrix S[e,j] = (dst_local[e]==j) is
0/1 and built with a single is_equal DVE op in bf16.

Sharding: nodes row-partitioned across 8 cores (12500/core); each edge owned
by the core owning its destination. Edges grouped by 128-node destination
block and 25000-row source chunk (dma_gather int16 indices), padded to
128-edge tiles; per-(block,chunk) tile counts are equalized across cores so
one SPMD program serves all 8. Gathers issue one dma_gather per (superblock
of 4 blocks, chunk) (~23 tiles = ~2900 indices per call); trailing pad slots
carry -1 indices which the Q7 gather kernel trims at runtime.

Phase A aggregates t1 node-major per block (psum += S^T @ Msg), scales by
dinv^2 on ScalarE, stores bf16 shard, AllGathers to the full t2 table.
Phase B aggregates t2 transposed (psum += Msg^T @ S), applies W12 via a
second matmul, then dinv scale (ScalarE) + bias + minmax + L2 normalize.
"""

import math

import numpy as np
import ml_dtypes

import concourse.bass as bass
import concourse.bacc as bacc
import concourse.mybir as mybir
import concourse.tile as tile
from concourse import bass_utils

NCORES = 8
BLK = 128
IN_C = 128
HID = 128
OUT_C = 64
CHUNK_ROWS = 25000  # dma_gather idx is int16: chunk-relative indices < 32768
SBN = 4  # destination blocks per gather superblock
MAXT = 8  # tiles per dma_gather call (1024 idx, 65 ring descs/lane: known-safe)
NQ = 4    # rotate dma_gather queue_num to decouple descgen from drain

BF16 = ml_dtypes.bfloat16

LAST_RESULTS = None
_PROGRAM_CACHE = {}


def _host_prep(x, edge_index, W1, b1, W2, b2):
    n = x.shape[0]
    assert n % NCORES == 0
    npc = n // NCORES
    nblk = math.ceil(npc / BLK)
    n_chunks = math.ceil(n / CHUNK_ROWS)

    src = edge_index[0].astype(np.int64)
    dst = edge_index[1].astype(np.int64)

    deg = (np.bincount(dst, minlength=n) + 1).astype(np.float32)
    dinv = (1.0 / np.sqrt(deg)).astype(np.float32)

    loop = np.arange(n, dtype=np.int64)
    s_all = np.concatenate([src, loop])
    d_all = np.concatenate([dst, loop])

    # rowsumA[d] = dinv[d] * sum_{e->d} dinv[src_e]  (self-loop included)
    acc = np.zeros(n, np.float32)
    np.add.at(acc, d_all, dinv[s_all])
    rowsumA = dinv * acc

    W12 = (W1.astype(np.float64) @ W2.astype(np.float64)).astype(np.float32)
    b1W2 = (b1 @ W2).astype(np.float32)
    bias_full = (rowsumA[:, None] * b1W2[None, :] + b2[None, :]).astype(np.float32)

    t1 = (x * dinv[:, None]).astype(BF16)

    core = d_all // npc
    within = d_all % npc
    blk = within // BLK
    colv = (within % BLK).astype(np.float32)
    chunk = s_all // CHUNK_ROWS
    rel_e = (s_all - chunk * CHUNK_ROWS).astype(np.int16)

    key = (core * nblk + blk) * n_chunks + chunk
    counts = np.bincount(key, minlength=NCORES * nblk * n_chunks).reshape(
        NCORES, nblk * n_chunks
    )
    # per-(block,chunk) slot count, equalized across cores (SPMD), packed
    # back-to-back within each (superblock, chunk) gather range (only the
    # range start is 128-aligned; boundary tiles span two blocks and get one
    # S-matrix per (tile, block) pair).
    m = counts.max(axis=0).reshape(nblk, n_chunks)
    n_sb = math.ceil(nblk / SBN)

    boff = np.zeros((nblk, n_chunks), np.int64)  # slot offset of block in range
    rc0 = np.zeros((n_sb, n_chunks), np.int64)   # global tile col of range
    rtl = np.zeros((n_sb, n_chunks), np.int64)   # tiles in range
    emap = {}                                    # (b, q, gcol) -> expanded col
    block_cols = [[] for _ in range(nblk)]       # per block: [(gcol, ecol), ...]
    cur = 0
    ecol_ct = 0
    for sbi in range(n_sb):
        blocks = range(sbi * SBN, min((sbi + 1) * SBN, nblk))
        for q in range(n_chunks):
            rc0[sbi, q] = cur
            off = 0
            for b in blocks:
                boff[b, q] = off
                off += int(m[b, q])
            rtl[sbi, q] = (off + BLK - 1) // BLK
            for b in blocks:
                if m[b, q] == 0:
                    continue
                lo_t = int(boff[b, q]) // BLK
                hi_t = int(boff[b, q] + m[b, q] - 1) // BLK
                for t in range(lo_t, hi_t + 1):
                    gcol = cur + t
                    emap[(b, q, gcol)] = ecol_ct
                    block_cols[b].append((gcol, ecol_ct))
                    ecol_ct += 1
            cur += int(rtl[sbi, q])
    t_total = cur
    e_total = ecol_ct

    order = np.argsort(key, kind="stable")
    ks = key[order]
    cs = colv[order]
    rel = rel_e[order]

    group_start = np.zeros(NCORES * nblk * n_chunks, np.int64)
    group_start[1:] = np.cumsum(counts.ravel())[:-1]
    r = np.arange(len(ks), dtype=np.int64) - group_start[ks]
    c_idx = ks // (nblk * n_chunks)
    b_idx = (ks // n_chunks) % nblk
    q_idx = ks % n_chunks
    s_rel = boff[b_idx, q_idx] + r
    gcol = rc0[b_idx // SBN, q_idx] + s_rel // BLK
    p_idx = s_rel % BLK
    ecol_arr = np.array(
        [emap[(b, q, g)] for b, q, g in zip(b_idx, q_idx, gcol)], dtype=np.int64
    )

    # int16 idx stream for dma_gather: index k of a call lives at
    # [k%16 (+16*replica), call_col0*8 + k//16]; range starts are 128-aligned
    # so this reduces to [p%16, gcol*8 + p//16] independent of call boundaries.
    srcs16 = np.zeros((NCORES, 16, t_total * 8), np.int16)
    dstf_arr = np.full((NCORES, BLK, e_total), -1.0, np.float32)
    srcs16[c_idx, p_idx % 16, gcol * 8 + p_idx // 16] = rel
    dstf_arr[c_idx, p_idx, ecol_arr] = cs

    # Pad slots keep idx 0 (safe in-bounds read; their dstf=-1 zeroes S).
    # NOTE: -1 trailing-trim is unusable under SPMD — the Q7 kernel trims by
    # value but the sequencer reserves ring space by num_idxs_reg, which is a
    # compile-time constant shared across cores; a mismatch drifts the ring
    # offsets against the SDMA tail and executes stale descriptors.
    srcs16 = np.tile(srcs16, (1, 8, 1))  # replicate for the 8 Q7 cores

    # per-node dinv / dinv^2 by (block-local row, block); bias by block
    pad_npc = nblk * BLK
    dinv_pad = np.zeros((NCORES, pad_npc), np.float32)
    dinv_pad[:, :npc] = dinv.reshape(NCORES, npc)
    dinvB = np.ascontiguousarray(
        dinv_pad.reshape(NCORES, nblk, BLK).transpose(0, 2, 1)
    )
    bias_pad = np.zeros((NCORES, pad_npc, OUT_C), np.float32)
    bias_pad[:, :npc] = bias_full.reshape(NCORES, npc, OUT_C)
    biasB = np.ascontiguousarray(
        bias_pad.reshape(NCORES, nblk, BLK, OUT_C).transpose(0, 2, 1, 3)
    ).reshape(NCORES, BLK, nblk * OUT_C)

    iota = np.tile(np.arange(BLK, dtype=np.float32), (BLK, 1)).astype(BF16)

    in_maps = []
    for c in range(NCORES):
        in_maps.append(
            {
                "t1": t1,
                "srcs16": np.ascontiguousarray(srcs16[c]),
                "dstf": np.ascontiguousarray(dstf_arr[c]),
                "dinvB": np.ascontiguousarray(dinvB[c]),
                "dinvB2": np.ascontiguousarray(dinvB[c] ** 2),
                "biasB": np.ascontiguousarray(biasB[c]),
                "iota": iota,
                "W12": np.ascontiguousarray(W12),
            }
        )
    return in_maps, (rc0, rtl, block_cols, t_total, e_total), npc, nblk, n_chunks


def _build_nc(n, npc, nblk, n_chunks, layout):
    rc0, rtl, block_cols, t_total, e_total = layout
    f32 = mybir.dt.float32
    bf16 = mybir.dt.bfloat16
    i16 = mybir.dt.int16
    n_sb = math.ceil(nblk / SBN)

    nc = bacc.Bacc(
        "TRN2",
        target_bir_lowering=False,
        debug=False,
        enable_asserts=False,
        num_devices=NCORES,
        num_swdge_queues=NQ,
    )

    t1 = nc.dram_tensor("t1", [n, IN_C], bf16, kind="ExternalInput").ap()
    srcs16 = nc.dram_tensor(
        "srcs16", [BLK, t_total * 8], i16, kind="ExternalInput"
    ).ap()
    dstf = nc.dram_tensor("dstf", [BLK, e_total], f32, kind="ExternalInput").ap()
    dinvB = nc.dram_tensor("dinvB", [BLK, nblk], f32, kind="ExternalInput").ap()
    dinvB2 = nc.dram_tensor("dinvB2", [BLK, nblk], f32, kind="ExternalInput").ap()
    biasB = nc.dram_tensor(
        "biasB", [BLK, nblk * OUT_C], f32, kind="ExternalInput"
    ).ap()
    iota = nc.dram_tensor("iota", [BLK, BLK], bf16, kind="ExternalInput").ap()
    W12 = nc.dram_tensor("W12", [IN_C, OUT_C], f32, kind="ExternalInput").ap()
    out = nc.dram_tensor("out", [npc, OUT_C], f32, kind="ExternalOutput").ap()

    ieq = mybir.AluOpType.is_equal
    mul = mybir.AluOpType.mult
    sub = mybir.AluOpType.subtract
    copyf = mybir.ActivationFunctionType.Copy

    def nb_of(b):
        return min(BLK, npc - b * BLK)

    def sb_blocks(sbi):
        return range(sbi * SBN, min((sbi + 1) * SBN, nblk))

    with tile.TileContext(nc) as tc:
        with (
            tc.tile_pool(name="dram", bufs=1, space="DRAM") as dram,
            tc.tile_pool(name="const", bufs=1) as constp,
            tc.tile_pool(name="meta", bufs=1) as metap,
            tc.tile_pool(name="msg", bufs=4) as msgp,
            tc.tile_pool(name="sel", bufs=12) as selp,
            tc.tile_pool(name="fin", bufs=6) as finp,
            tc.tile_pool(name="stat", bufs=8) as statp,
            tc.tile_pool(name="psA", bufs=6, space="PSUM") as psA,
            tc.tile_pool(name="psB", bufs=2, space="PSUM") as psB,
        ):
            g_shard = dram.tile([npc, IN_C], bf16)
            g_full = dram.tile([n, IN_C], bf16, addr_space="Shared")

            iotas = constp.tile([BLK, BLK], bf16)
            nc.sync.dma_start(out=iotas[:], in_=iota)
            W12s = constp.tile([IN_C, OUT_C], f32)
            nc.sync.dma_start(out=W12s[:], in_=W12)
            dinvBs = constp.tile([BLK, nblk], f32)
            nc.sync.dma_start(out=dinvBs[:], in_=dinvB)
            dinvB2s = constp.tile([BLK, nblk], f32)
            nc.sync.dma_start(out=dinvB2s[:], in_=dinvB2)
            biasBs = constp.tile([BLK, nblk * OUT_C], f32)
            nc.sync.dma_start(out=biasBs[:], in_=biasB)
            dstf_s = metap.tile([BLK, e_total], f32)
            nc.sync.dma_start(out=dstf_s[:], in_=dstf)
            srcs16_s = metap.tile([BLK, t_total * 8], i16)
            nc.sync.dma_start(out=srcs16_s[:], in_=srcs16)

            qrot = [0]

            def gather_sb(sbi, table_full, elem, msg_tag):
                """One superblock's gathers, queue-rotated across calls."""
                sb_col0 = int(rc0[sbi, 0])
                t_sb = int(rtl[sbi].sum())
                msg = msgp.tile([BLK, t_sb * elem], bf16, tag=msg_tag)
                for q in range(n_chunks):
                    cs = int(rc0[sbi, q])
                    tq = int(rtl[sbi, q])
                    if tq == 0:
                        continue
                    hi_r = min((q + 1) * CHUNK_ROWS, n)
                    for k in range(0, tq, MAXT):
                        tk = min(MAXT, tq - k)
                        lo = cs - sb_col0 + k
                        nc.gpsimd.dma_gather(
                            out_ap=msg[:, lo * elem : (lo + tk) * elem].rearrange(
                                "p (t e) -> p t e", e=elem
                            ),
                            in_ap=table_full[q * CHUNK_ROWS : hi_r, :],
                            idxs_ap=srcs16_s[:, (cs + k) * 8 : (cs + k + tk) * 8],
                            num_idxs=tk * BLK,
                            num_idxs_reg=tk * BLK,
                            elem_size=elem,
                            queue_num=qrot[0],
                        )
                        qrot[0] = (qrot[0] + 1) % NQ
                return msg, sb_col0

            # ---- Phase A: g = dinv^2 * sum_e t1[src] per dst block ----
            for sbi in range(n_sb):
                msg, sb_col0 = gather_sb(sbi, t1, IN_C, "msg")
                for b in sb_blocks(sbi):
                    nb = nb_of(b)
                    cols = block_cols[b]
                    ps = psA.tile([BLK, IN_C], f32, tag="psA")
                    for j, (g, ec) in enumerate(cols):
                        S = selp.tile([BLK, BLK], bf16, tag="S")
                        nc.vector.tensor_scalar(
                            out=S[:],
                            in0=iotas[:],
                            scalar1=dstf_s[:, ec : ec + 1],
                            scalar2=None,
                            op0=ieq,
                        )
                        lo = g - sb_col0
                        nc.tensor.matmul(
                            out=ps[:nb, :],
                            lhsT=S[:, :nb],
                            rhs=msg[:, lo * IN_C : (lo + 1) * IN_C],
                            start=(j == 0),
                            stop=(j == len(cols) - 1),
                        )
                    gt = finp.tile([BLK, IN_C], bf16, tag="gt")
                    nc.scalar.activation(
                        gt[:nb, :], ps[:nb, :], copyf,
                        scale=dinvB2s[:nb, b : b + 1],
                    )
                    nc.sync.dma_start(
                        out=g_shard[b * BLK : b * BLK + nb, :], in_=gt[:nb, :]
                    )

            nc.gpsimd.collective_compute(
                "AllGather",
                mybir.AluOpType.bypass,
                replica_groups=[list(range(NCORES))],
                ins=[g_shard[:]],
                outs=[g_full[:]],
            )

            # ---- Phase B: z = dinv * (sum_e t2[src]) @ W12 + bias; normalize ----
            for sbi in range(n_sb):
                msg2, sb_col0 = gather_sb(sbi, g_full, IN_C, "msg")
                for b in sb_blocks(sbi):
                    nb = nb_of(b)
                    cols = block_cols[b]
                    psT = psA.tile([IN_C, BLK], f32, tag="psA")
                    for j, (g, ec) in enumerate(cols):
                        S = selp.tile([BLK, BLK], bf16, tag="S")
                        nc.vector.tensor_scalar(
                            out=S[:],
                            in0=iotas[:],
                            scalar1=dstf_s[:, ec : ec + 1],
                            scalar2=None,
                            op0=ieq,
                        )
                        lo = g - sb_col0
                        nc.tensor.matmul(
                            out=psT[:, :nb],
                            lhsT=msg2[:, lo * IN_C : (lo + 1) * IN_C],
                            rhs=S[:, :nb],
                            start=(j == 0),
                            stop=(j == len(cols) - 1),
                        )
                    a2 = finp.tile([IN_C, BLK], f32, tag="a2")
                    nc.scalar.copy(a2[:, :nb], psT[:, :nb])
                    zps = psB.tile([BLK, OUT_C], f32, tag="psB")
                    nc.tensor.matmul(
                        out=zps[:nb, :],
                        lhsT=a2[:, :nb],
                        rhs=W12s[:],
                        start=True,
                        stop=True,
                    )
                    z = finp.tile([BLK, OUT_C], f32, tag="z")
                    nc.scalar.activation(
                        z[:nb, :], zps[:nb, :], copyf,
                        scale=dinvBs[:nb, b : b + 1],
                    )
                    z2 = finp.tile([BLK, OUT_C], f32, tag="z2")
                    nc.vector.tensor_add(
                        z2[:nb, :], z[:nb, :],
                        biasBs[:nb, b * OUT_C : b * OUT_C + OUT_C],
                    )
                    zmax = statp.tile([BLK, 1], f32, tag="zmax")
                    nc.vector.tensor_reduce(
                        zmax[:nb], z2[:nb, :], axis=mybir.AxisListType.X,
                        op=mybir.AluOpType.max,
                    )
                    zmin = statp.tile([BLK, 1], f32, tag="zmin")
                    nc.vector.tensor_reduce(
                        zmin[:nb], z2[:nb, :], axis=mybir.AxisListType.X,
                        op=mybir.AluOpType.min,
                    )
                    rng_t = statp.tile([BLK, 1], f32, tag="rng")
                    nc.vector.tensor_sub(rng_t[:nb], zmax[:nb], zmin[:nb])
                    rinv = statp.tile([BLK, 1], f32, tag="rinv")
                    nc.vector.reciprocal(rinv[:nb], rng_t[:nb])
                    zs = finp.tile([BLK, OUT_C], f32, tag="zs")
                    nc.vector.tensor_scalar(
                        out=zs[:nb, :],
                        in0=z2[:nb, :],
                        scalar1=zmin[:nb],
                        scalar2=rinv[:nb],
                        op0=sub,
                        op1=mul,
                    )
                    sq = finp.tile([BLK, OUT_C], f32, tag="sq")
                    ssq = statp.tile([BLK, 1], f32, tag="ssq")
                    nc.scalar.activation(
                        sq[:nb, :],
                        zs[:nb, :],
                        mybir.ActivationFunctionType.Square,
                        accum_out=ssq[:nb],
                    )
                    snrm = statp.tile([BLK, 1], f32, tag="snrm")
                    nc.scalar.sqrt(snrm[:nb], ssq[:nb])
                    # no eps clamp: minmax scaling puts a 1.0 in every row of
                    # zs, so ssq >= 1 and the 1e-12 guard can never bind
                    ninv = statp.tile([BLK, 1], f32, tag="ninv")
                    nc.vector.reciprocal(ninv[:nb], snrm[:nb])
                    res = finp.tile([BLK, OUT_C], f32, tag="res")
                    nc.vector.tensor_scalar_mul(res[:nb, :], zs[:nb, :], ninv[:nb])
                    nc.sync.dma_start(
                        out=out[b * BLK : b * BLK + nb, :], in_=res[:nb, :]
                    )

    nc.compile()
    return nc


def kernel(x, edge_index, W1, b1, W2, b2, trace=False):
    global LAST_RESULTS
    x = np.asarray(x)
    edge_index = np.asarray(edge_index)
    W1 = np.asarray(W1, dtype=np.float32)
    b1 = np.asarray(b1, dtype=np.float32)
    W2 = np.asarray(W2, dtype=np.float32)
    b2 = np.asarray(b2, dtype=np.float32)

    n = x.shape[0]
    in_maps, layout, npc, nblk, n_chunks = _host_prep(
        x, edge_index, W1, b1, W2, b2
    )

    key = (n, layout[0].tobytes(), layout[1].tobytes())
    nc = _PROGRAM_CACHE.get(key)
    if nc is None:
        nc = _build_nc(n, npc, nblk, n_chunks, layout)
        _PROGRAM_CACHE[key] = nc

    results = bass_utils.run_bass_kernel_spmd(
        nc, in_maps, core_ids=list(range(NCORES)), trace=trace
    )
    LAST_RESULTS = results
    return np.concatenate([results.results[c]["out"] for c in range(NCORES)], axis=0)
